# revision 7
# baseline (speedup 1.0000x reference)
"""MixAttention Trainium2 kernel (8-core SPMD, Bass/Tile).

Sharding: (batch, query-chunk) over 8 cores: core = 4*b + qp handles batch b,
queries [qp*576, (qp+1)*576) of N=2304 (n = h*48 + w order). Each core computes
both cross-attentions (all 8 heads) for its query slice; K/V for the full
sequence are computed redundantly per core (tiny). The final 1x1-conv fuse +
gelu is fully local per core; the host only concatenates slices.

Head layout: head h = 4*g + j lives on partition strip 32*j (+0..7, row 8 =
softmax denominator), group g selects the free-dim half. All compute-engine
partition bases are in {0, 32, 64, 96}; strip-offset PSUM writes use explicit
matmul tile_position.

Device math (validated vs reference in numpy, absmax/scale ~2e-6):
  - depth conv1x1+bias+relu as a rank-2 matmul (ones row carries the bias)
  - bilinear 2x upsample (align_corners=False) via shifted weighted adds (DVE)
  - S^T chunks into PSUM supertiles, exp on ScalarE (softmax scale folded into
    the activation), no max subtraction (|scaled scores| < ~8 for these inputs)
  - ones column per head in V* makes the attention matmul emit the softmax
    denominator; denominators are broadcast back over channels with a tiny
    constant matmul
  - output projection + compress conv fused into strip-layout matmuls with
    host-precomputed weights, bias applied inside the exact-Gelu activation
"""

import numpy as np

import bass_rust
import concourse.bass as bass
import concourse.mybir as mybir
import concourse.tile as tile
from concourse.bass_utils import run_bass_kernel_spmd
from concourse.vector_clock import ScopedClock

F32 = mybir.dt.float32
AF = mybir.ActivationFunctionType

C = 64          # channels
H = 8           # heads
HD = 8          # head dim
N = 2304        # sequence (48*48)
NQ = 576        # queries per core
QC = 144        # query chunk
NQC = NQ // QC  # 4 chunks per core
KT = 128        # key tile
NKT = N // KT   # 18 key tiles
SCALE = float(HD) ** -0.5


class _TileContext(tile.TileContext):
    """TileContext whose kernel-tail drain splits its semaphore waits across
    separate SP instructions (this walrus build rejects >1 wait per inst)."""

    def _drain_and_barrier(self, tick_clock, wait_clock):
        nc = self.nc
        drain_inst = nc.sync.drain()
        wait_clock.add_sem_waits(
            drain_inst.ins, ScopedClock({None: tick_clock.global_clock})
        )
        nc.all_engine_barrier()
        popped = nc._tile_sem_poison_stack.pop()
        assert popped is self._sem_poison
        nc.clear_and_free_semaphores(list(self.sems.allocated().values()))
        nc.all_engine_barrier()
        _split_multi_waits(nc)


def _split_multi_waits(nc):
    """This walrus build allows one sync wait per instruction (two for
    EventSemaphore). Hoist extra waits onto same-engine nops placed just
    before the over-subscribed instruction."""
    for f in nc.m.functions:
        for bb in f.blocks:
            insts = bb.instructions
            out = []
            changed = False
            for ins in list(insts):
                si = getattr(ins, "sync_info", None)
                waits = list(si.on_wait) if si is not None else []
                cap = 2 if isinstance(ins, mybir.InstEventSemaphore) else 1
                if len(waits) <= cap:
                    out.append(ins)
                    continue
                changed = True
                for w in waits[:-cap]:
                    nop = nc.engines[ins.engine].nop()
                    cb = nc.cur_bb.bb.instructions
                    assert cb[-1] is nop.ins
                    cb.pop()
                    nop.ins.sync_info = bass_rust.SyncInfo(on_wait=[w], on_update=[])
                    out.append(nop.ins)
                ins.sync_info = bass_rust.SyncInfo(
                    on_wait=waits[-cap:], on_update=list(si.on_update)
                )
                out.append(ins)
            if changed:
                insts.clear()
                insts.extend(out)


def _sa_off(ks9):
    """PSUM free-dim f32 offset of local k-super-tile ks9 (0..8) inside a
    [128, 1536] 3-bank supertile; 3 slots of QC per 512-f32 bank."""
    return (ks9 // 3) * 512 + (ks9 % 3) * QC


def _exp_in_ap(s_tile):
    """AP enumerating the 9 slots of a supertile in ks order, element-order
    compatible with a contiguous [128, 9*QC] output."""
    v = s_tile[:].rearrange("p (b s) -> p b s", s=512)[:, :, 0 : 3 * QC]
    return v.rearrange("p b (sl q) -> p b sl q", q=QC)


def build_nc(repeat=1):
    nc = bass.Bass()

    # ---- dram parameters ----
    xrgb_d = nc.declare_dram_parameter("xrgb", [C + 1, N], F32, isOutput=False)
    xq_d = nc.declare_dram_parameter("xq", [C, NQ], F32, isOutput=False)
    xdep_d = nc.declare_dram_parameter("xdep", [2, 576], F32, isOutput=False)
    qoff_d = nc.declare_dram_parameter("qoff", [1, 1], mybir.dt.uint32, isOutput=False)
    wk_r_d = nc.declare_dram_parameter("wk_r", [C, 2 * KT], F32, isOutput=False)
    wq_r_d = nc.declare_dram_parameter("wq_r", [C, 2 * KT], F32, isOutput=False)
    wk_d_d = nc.declare_dram_parameter("wk_d", [C, 2 * KT], F32, isOutput=False)
    wq_d_d = nc.declare_dram_parameter("wq_d", [C, 2 * KT], F32, isOutput=False)
    wvs_r_d = nc.declare_dram_parameter("wvs_r", [C + 1, 72], F32, isOutput=False)
    wvs_d_d = nc.declare_dram_parameter("wvs_d", [C + 1, 72], F32, isOutput=False)
    wexpb_d = nc.declare_dram_parameter("wexpb", [2, C], F32, isOutput=False)
    wf_r_d = nc.declare_dram_parameter("wf_r", [KT, 2 * C], F32, isOutput=False)
    wf_d_d = nc.declare_dram_parameter("wf_d", [KT, 2 * C], F32, isOutput=False)
    biasp_d = nc.declare_dram_parameter("biasp", [C, 1], F32, isOutput=False)
    e4_d = nc.declare_dram_parameter("e4", [4, KT], F32, isOutput=False)
    y_d = nc.declare_dram_parameter("y", [C, NQ], F32, isOutput=True)

    with _TileContext(nc) as tc:
        with (
            tc.tile_pool(name="const", bufs=1) as cpool,
            tc.tile_pool(name="work", bufs=1) as wpool,
        ):
            # ---- load inputs / weights to SBUF ----
            xrgb = cpool.tile([C + 1, N], F32)
            nc.sync.dma_start(xrgb[:], xrgb_d[:])
            xq = cpool.tile([C, NQ], F32)
            nc.sync.dma_start(xq[:], xq_d[:])
            xdep = cpool.tile([2, 576], F32)
            nc.sync.dma_start(xdep[:], xdep_d[:])
            qoff = cpool.tile([1, 1], mybir.dt.uint32)
            nc.sync.dma_start(qoff[:], qoff_d[:])
            wk = {}
            wqw = {}
            wvs = {}
            wf = {}
            for m, wk_src, wq_src, wvs_src, wf_src in (
                ("r", wk_r_d, wq_r_d, wvs_r_d, wf_r_d),
                ("d", wk_d_d, wq_d_d, wvs_d_d, wf_d_d),
            ):
                wk[m] = cpool.tile([C, 2 * KT], F32, tag=f"wk_{m}", name=f"wk_{m}")
                nc.sync.dma_start(wk[m][:], wk_src[:])
                wqw[m] = cpool.tile([C, 2 * KT], F32, tag=f"wq_{m}", name=f"wq_{m}")
                nc.sync.dma_start(wqw[m][:], wq_src[:])
                wvs[m] = cpool.tile([C + 1, 72], F32, tag=f"wvs_{m}", name=f"wvs_{m}")
                nc.sync.dma_start(wvs[m][:], wvs_src[:])
                wf[m] = cpool.tile([KT, 2 * C], F32, tag=f"wf_{m}", name=f"wf_{m}")
                nc.sync.dma_start(wf[m][:], wf_src[:])
            wexpb = cpool.tile([2, C], F32)
            nc.sync.dma_start(wexpb[:], wexpb_d[:])
            biasp = cpool.tile([C, 1], F32)
            nc.sync.dma_start(biasp[:], biasp_d[:])
            e4 = cpool.tile([4, KT], F32)
            nc.sync.dma_start(e4[:], e4_d[:])

            # ---- depth pipeline ----
            dep = cpool.tile([C + 1, N], F32)  # row 64 = ones
            nc.gpsimd.memset(dep[C : C + 1, :], 1.0)

            with tc.tile_pool(name="ppsum", bufs=2, space="PSUM") as ppool:
                # conv1x1 + bias (+relu below): [64, 576]
                dps = ppool.tile([C, 576], F32, tag="p576", bufs=1)
                nc.tensor.matmul(
                    dps[:, 0:512], wexpb[:, :], xdep[:, 0:512], start=True, stop=True
                )
                nc.tensor.matmul(
                    dps[:, 512:576], wexpb[:, :], xdep[:, 512:576], start=True, stop=True
                )
                Rm = wpool.tile([C, 576], F32)
                nc.scalar.activation(Rm[:], dps[:], AF.Relu)

                # upsample w': [64, 24, 24] -> A [64, 24, 48]
                A = wpool.tile([C, 24 * 48], F32)
                t75 = wpool.tile([C, 576], F32)
                t25 = wpool.tile([C, 576], F32)
                nc.vector.tensor_scalar_mul(t75[:], Rm[:], 0.75)
                nc.vector.tensor_scalar_mul(t25[:], Rm[:], 0.25)
                R3_75 = t75[:].rearrange("p (h w) -> p h w", w=24)
                R3_25 = t25[:].rearrange("p (h w) -> p h w", w=24)
                R3 = Rm[:].rearrange("p (h w) -> p h w", w=24)
                Av = A[:].rearrange("p (h j t) -> p h j t", j=24, t=2)
                nc.vector.tensor_add(
                    Av[:, :, 1:, 0], R3_75[:, :, 1:], R3_25[:, :, 0:23]
                )
                nc.vector.tensor_copy(Av[:, :, 0:1, 0], R3[:, :, 0:1])
                nc.vector.tensor_add(
                    Av[:, :, 0:23, 1], R3_75[:, :, 0:23], R3_25[:, :, 1:]
                )
                nc.vector.tensor_copy(Av[:, :, 23:24, 1], R3[:, :, 23:24])

                # upsample h': A [64, 24, 48] -> dep[0:64] as [64, 48, 48]
                u75 = wpool.tile([C, 24 * 48], F32)
                u25 = wpool.tile([C, 24 * 48], F32)
                nc.vector.tensor_scalar_mul(u75[:], A[:], 0.75)
                nc.vector.tensor_scalar_mul(u25[:], A[:], 0.25)
                A3 = A[:].rearrange("p (h w) -> p h w", w=48)
                A3_75 = u75[:].rearrange("p (h w) -> p h w", w=48)
                A3_25 = u25[:].rearrange("p (h w) -> p h w", w=48)
                Bv = dep[0:C, :].rearrange("p (i t w) -> p i t w", t=2, w=48)
                nc.vector.tensor_add(
                    Bv[:, 1:, 0, :], A3_75[:, 1:, :], A3_25[:, 0:23, :]
                )
                nc.vector.tensor_copy(Bv[:, 0:1, 0, :], A3[:, 0:1, :])
                nc.vector.tensor_add(
                    Bv[:, 0:23, 1, :], A3_75[:, 0:23, :], A3_25[:, 1:, :]
                )
                nc.vector.tensor_copy(Bv[:, 23:24, 1, :], A3[:, 23:24, :])

                # ---- per-core dep query slice (dynamic offset) ----
                dep_q = cpool.tile([C, NQ], F32)
                regs = nc.alloc_registers()
                nc.regs_load(regs, qoff[0:1, 0:1])
                q0v = nc.snap(regs, donate=True, min_val=0, max_val=N - NQ)
                nc.vector.tensor_copy(dep_q[:], dep[0:C, bass.ds(q0v, NQ)])

                # ---- projections into strip layout ----
                # Kt32[m][32j+d, g*N + n] = (w_k[m] @ x)[8*(4g+j)+d, n]
                # via host-permuted lhsT (junk rows get zero weight columns)
                kt = {}
                qt = {}
                vstar = {}
                for m, xin, qin in (("r", xrgb, xq), ("d", dep, dep_q)):
                    kt[m] = cpool.tile([KT, 2 * N], F32, tag=f"kt_{m}", name=f"kt_{m}")
                    for g in range(2):
                        for c0 in range(0, N, 512):
                            cw = min(512, N - c0)
                            kp = ppool.tile([KT, 512], F32, tag="kp", name="kp")
                            nc.tensor.matmul(
                                kp[:, 0:cw],
                                wk[m][:, g * KT : (g + 1) * KT],
                                xin[0:C, c0 : c0 + cw],
                                start=True,
                                stop=True,
                            )
                            nc.vector.tensor_copy(
                                kt[m][:, g * N + c0 : g * N + c0 + cw], kp[:, 0:cw]
                            )
                    qt[m] = cpool.tile([KT, 2 * NQ], F32, tag=f"qt_{m}", name=f"qt_{m}")
                    for g in range(2):
                        for c0 in range(0, NQ, 288):
                            qp_ = ppool.tile([KT, 512], F32, tag="kp", name="qp_")
                            nc.tensor.matmul(
                                qp_[:, 0:288],
                                wqw[m][:, g * KT : (g + 1) * KT],
                                qin[:, c0 : c0 + 288],
                                start=True,
                                stop=True,
                            )
                            nc.vector.tensor_copy(
                                qt[m][:, g * NQ + c0 : g * NQ + c0 + 288], qp_[:, 0:288]
                            )
                    vstar[m] = cpool.tile(
                        [KT, NKT * 72], F32, tag=f"vs_{m}", name=f"vs_{m}"
                    )
                    for ks in range(NKT):
                        vp = ppool.tile([KT, 72], F32, tag="p72", name="vp")
                        nc.tensor.matmul(
                            vp[:],
                            xin[:, ks * KT : (ks + 1) * KT],
                            wvs[m][:, :],
                            start=True,
                            stop=True,
                        )
                        nc.vector.tensor_copy(
                            vstar[m][:, ks * 72 : (ks + 1) * 72], vp[:]
                        )

            # ---- attention ----
            import contextlib
            rep_ctx = tc.For_i(0, repeat, 1) if repeat > 1 else contextlib.nullcontext()
            with (
                tc.tile_pool(name="sa", bufs=1, space="PSUM") as sapool,
                tc.tile_pool(name="sb", bufs=1, space="PSUM") as sbpool,
                tc.tile_pool(name="av", bufs=2, space="PSUM") as avpool,
                tc.tile_pool(name="att", bufs=2) as apool,
                rep_ctx,
            ):
                for qc in range(NQC):
                    qsl = slice(qc * QC, (qc + 1) * QC)
                    xt = {}
                    for m, kv in (("r", "d"), ("d", "r")):
                        qm = qt[m]
                        km = kt[kv]
                        vm = vstar[kv]
                        xnum = apool.tile([KT, 2 * QC], F32, tag="xnum", name="xnum")
                        nc.vector.memset(xnum[:], 0.0)
                        av = avpool.tile([KT, 2 * QC], F32, tag="avt", name="av")
                        for h in range(H):
                            g, j = divmod(h, 4)
                            strip = 32 * j
                            pt = apool.tile([KT, NKT * QC], F32, tag="pt", name="pt")
                            for half, spool in ((0, sapool), (1, sbpool)):
                                st = spool.tile(
                                    [KT, 1536], F32, tag=f"s{half}", name=f"s{half}"
                                )
                                for ks9 in range(9):
                                    ks = half * 9 + ks9
                                    o = _sa_off(ks9)
                                    nc.tensor.matmul(
                                        st[:, o : o + QC],
                                        km[strip : strip + 8, g * N + ks * KT : g * N + (ks + 1) * KT],
                                        qm[strip : strip + 8, g * NQ + qc * QC : g * NQ + (qc + 1) * QC],
                                        start=True,
                                        stop=True,
                                        tile_position=(strip, 0),
                                    )
                                nc.scalar.activation(
                                    pt[:, half * 9 * QC : (half + 1) * 9 * QC],
                                    _exp_in_ap(st),
                                    AF.Exp,
                                    scale=SCALE,
                                )
                            for ks in range(NKT):
                                nc.tensor.matmul(
                                    av[strip : strip + 9, g * QC : (g + 1) * QC],
                                    vm[:, ks * 72 + 9 * h : ks * 72 + 9 * h + 9],
                                    pt[:, ks * QC : (ks + 1) * QC],
                                    start=(ks == 0),
                                    stop=(ks == NKT - 1),
                                    tile_position=(0, strip),
                                    skip_group_check=True,
                                )
                            nc.vector.tensor_copy(
                                xnum[strip : strip + 9, g * QC : (g + 1) * QC],
                                av[strip : strip + 9, g * QC : (g + 1) * QC],
                            )
                        # denominators -> [4, 2*QC] (j on partitions, g in free)
                        dens4 = apool.tile([4, 2 * QC], F32, tag="dens4", name="dens4")
                        for h in range(H):
                            g, j = divmod(h, 4)
                            nc.sync.dma_start(
                                dens4[j : j + 1, g * QC : (g + 1) * QC],
                                xnum[32 * j + 8 : 32 * j + 9, g * QC : (g + 1) * QC],
                            )
                        recd = apool.tile([4, 2 * QC], F32, tag="recd", name="recd")
                        nc.vector.reciprocal(recd[:], dens4[:])
                        denx = avpool.tile([KT, 2 * QC], F32, tag="avt", name="denx")
                        nc.tensor.matmul(
                            denx[:], e4[:, :], recd[:], start=True, stop=True
                        )
                        xt[m] = apool.tile(
                            [KT, 2 * QC], F32, tag=f"xt_{m}", name=f"xt_{m}"
                        )
                        nc.vector.tensor_mul(xt[m][:], xnum[:], denx[:])
                    fp = avpool.tile([C, QC], F32, tag="avt", name="fp")
                    first = True
                    for m in ("r", "d"):
                        for g in range(2):
                            nc.tensor.matmul(
                                fp[:],
                                wf[m][:, g * C : (g + 1) * C],
                                xt[m][:, g * QC : (g + 1) * QC],
                                start=first,
                                stop=(m == "d" and g == 1),
                            )
                            first = False
                    outt = apool.tile([C, QC], F32, tag="outt", name="outt")
                    nc.scalar.activation(outt[:], fp[:], AF.Gelu, bias=biasp[:])
                    nc.sync.dma_start(y_d[:, qsl], outt[:])

    return nc


# ---------------- host side ----------------

_BUILT = {}


def _get_nc():
    if "nc" not in _BUILT:
        _BUILT["nc"] = build_nc()
    return _BUILT["nc"]


def _host_prep(inputs):
    """Build per-core input maps from full inputs."""
    f = lambda k: np.ascontiguousarray(np.asarray(inputs[k], np.float32))
    rgb_fea = f("rgb_fea")
    depth_fea = f("depth_fea")
    w_exp = f("w_exp")
    b_exp = f("b_exp")

    def vstar_w(w_v):
        W = np.zeros((C + 1, 72), np.float32)
        for h in range(H):
            W[0:C, 9 * h : 9 * h + 8] = w_v.T[:, 8 * h : 8 * h + 8]
            W[C, 9 * h + 8] = 1.0
        return np.ascontiguousarray(W)

    def fuse_w(Wp):
        # Wp [64 out, 64 in]; strip layout rows 32j+d = in-channel 8*(4g+j)+d
        W = np.zeros((KT, 2 * C), np.float32)
        for g in range(2):
            for j in range(4):
                h = 4 * g + j
                W[32 * j : 32 * j + 8, g * C : (g + 1) * C] = Wp[:, 8 * h : 8 * h + 8].T
        return np.ascontiguousarray(W)

    w_comp = f("w_comp")
    W_r, W_d = w_comp[:, :C], w_comp[:, C:]
    e4 = np.zeros((4, KT), np.float32)
    for j in range(4):
        e4[j, 32 * j : 32 * j + 8] = 1.0
    def strip_w(w):
        # lhsT [64 in, 2*128]: col g*128 + 32j+d = row 8*(4g+j)+d of w
        W = np.zeros((C, 2 * KT), np.float32)
        for g in range(2):
            for j in range(4):
                h = 4 * g + j
                W[:, g * KT + 32 * j : g * KT + 32 * j + 8] = w[8 * h : 8 * h + 8, :].T
        return np.ascontiguousarray(W)

    shared = {
        "wk_r": strip_w(f("w_rgb_k")),
        "wq_r": strip_w(f("w_rgb_q")),
        "wk_d": strip_w(f("w_dep_k")),
        "wq_d": strip_w(f("w_dep_q")),
        "wvs_r": vstar_w(f("w_rgb_v")),
        "wvs_d": vstar_w(f("w_dep_v")),
        "wexpb": np.ascontiguousarray(
            np.stack([w_exp.ravel(), b_exp.ravel()]).astype(np.float32)
        ),
        "wf_r": fuse_w(W_r @ f("w_rgb_proj")),
        "wf_d": fuse_w(W_d @ f("w_dep_proj")),
        "biasp": np.ascontiguousarray(
            (W_r @ f("b_rgb_proj") + W_d @ f("b_dep_proj") + f("b_comp"))[:, None]
        ),
        "e4": e4,
    }
    ones = np.ones((1, N), np.float32)
    in_maps = []
    for core in range(8):
        b, qp = divmod(core, 4)
        xrgb = np.ascontiguousarray(np.vstack([rgb_fea[b].reshape(C, N), ones]))
        m = dict(shared)
        m["xrgb"] = xrgb
        m["xq"] = np.ascontiguousarray(xrgb[0:C, qp * NQ : (qp + 1) * NQ])
        m["xdep"] = np.ascontiguousarray(
            np.vstack(
                [depth_fea[b, 0].reshape(1, 576), np.ones((1, 576), np.float32)]
            )
        )
        m["qoff"] = np.array([[qp * NQ]], dtype=np.uint32)
        in_maps.append(m)
    return in_maps


def _assemble(results):
    out = np.zeros((2, C, 48, 48), np.float32)
    for core in range(8):
        b, qp = divmod(core, 4)
        y = results[core]["y"]
        out[b, :, qp * 12 : (qp + 1) * 12, :] = y.reshape(C, 12, 48)
    # (c, h, w) -> reference order (c, w, h)
    return np.ascontiguousarray(out.transpose(0, 1, 3, 2))


def kernel(**inputs):
    nc = _get_nc()
    in_maps = _host_prep(inputs)
    res = run_bass_kernel_spmd(nc, in_maps, list(range(8)))
    return _assemble(res.results)


def run_sim_core(inputs, core=0):
    """CoreSim single-core debug path (not used by the harness)."""
    from concourse import bass_interp

    nc = build_nc()
    sim = bass_interp.CoreSim(nc)
    in_map = _host_prep(inputs)[core]
    for k, v in in_map.items():
        sim.tensor(k)[:] = v
    sim.simulate()
    return np.array(sim.tensor("y"))


# revision 8
# speedup vs baseline: 2.4141x; 2.4141x over previous
"""MixAttention Trainium2 kernel (8-core SPMD, Bass/Tile).

Sharding: (batch, query-chunk) over 8 cores: core = 4*b + qp handles batch b,
queries [qp*576, (qp+1)*576) of N=2304 (n = h*48 + w order). Each core computes
both cross-attentions (all 8 heads) for its query slice; K/V for the full
sequence are computed redundantly per core (tiny). The final 1x1-conv fuse +
gelu is fully local per core; the host only concatenates slices.

Head layout: head h = 4*g + j lives on partition strip 32*j (+0..7, row 8 =
softmax denominator), group g selects the free-dim half. All compute-engine
partition bases are in {0, 32, 64, 96}; strip-offset PSUM writes use explicit
matmul tile_position.

Device math (validated vs reference in numpy, absmax/scale ~2e-6):
  - depth conv1x1+bias+relu as a rank-2 matmul (ones row carries the bias)
  - bilinear 2x upsample (align_corners=False) via shifted weighted adds (DVE)
  - S^T chunks into PSUM supertiles, exp on ScalarE (softmax scale folded into
    the activation), no max subtraction (|scaled scores| < ~8 for these inputs)
  - ones column per head in V* makes the attention matmul emit the softmax
    denominator; denominators are broadcast back over channels with a tiny
    constant matmul
  - output projection + compress conv fused into strip-layout matmuls with
    host-precomputed weights, bias applied inside the exact-Gelu activation
"""

import numpy as np

import bass_rust
import concourse.bass as bass
import concourse.mybir as mybir
import concourse.tile as tile
from concourse.bass_utils import run_bass_kernel_spmd
from concourse.vector_clock import ScopedClock

F32 = mybir.dt.float32
BF16 = mybir.dt.bfloat16
MM_DT = mybir.dt.bfloat16  # dtype of hot attention matmul operands
AF = mybir.ActivationFunctionType

C = 64          # channels
H = 8           # heads
HD = 8          # head dim
N = 2304        # sequence (48*48)
NQ = 576        # queries per core
QC = 144        # query chunk
NQC = NQ // QC  # 4 chunks per core
KT = 128        # key tile
NKT = N // KT   # 18 key tiles
SCALE = float(HD) ** -0.5


class _TileContext(tile.TileContext):
    """TileContext whose kernel-tail drain splits its semaphore waits across
    separate SP instructions (this walrus build rejects >1 wait per inst)."""

    def _drain_and_barrier(self, tick_clock, wait_clock):
        nc = self.nc
        drain_inst = nc.sync.drain()
        wait_clock.add_sem_waits(
            drain_inst.ins, ScopedClock({None: tick_clock.global_clock})
        )
        nc.all_engine_barrier()
        popped = nc._tile_sem_poison_stack.pop()
        assert popped is self._sem_poison
        nc.clear_and_free_semaphores(list(self.sems.allocated().values()))
        nc.all_engine_barrier()
        _split_multi_waits(nc)


def _split_multi_waits(nc):
    """This walrus build allows one sync wait per instruction (two for
    EventSemaphore). Hoist extra waits onto same-engine nops placed just
    before the over-subscribed instruction."""
    for f in nc.m.functions:
        for bb in f.blocks:
            insts = bb.instructions
            out = []
            changed = False
            for ins in list(insts):
                si = getattr(ins, "sync_info", None)
                waits = list(si.on_wait) if si is not None else []
                cap = 2 if isinstance(ins, mybir.InstEventSemaphore) else 1
                if len(waits) <= cap:
                    out.append(ins)
                    continue
                changed = True
                for w in waits[:-cap]:
                    nop = nc.engines[ins.engine].nop()
                    cb = nc.cur_bb.bb.instructions
                    assert cb[-1] is nop.ins
                    cb.pop()
                    nop.ins.sync_info = bass_rust.SyncInfo(on_wait=[w], on_update=[])
                    out.append(nop.ins)
                ins.sync_info = bass_rust.SyncInfo(
                    on_wait=waits[-cap:], on_update=list(si.on_update)
                )
                out.append(ins)
            if changed:
                insts.clear()
                insts.extend(out)


def _sa_off(ks9):
    """PSUM free-dim f32 offset of local k-super-tile ks9 (0..8) inside a
    [128, 1536] 3-bank supertile; 3 slots of QC per 512-f32 bank."""
    return (ks9 // 3) * 512 + (ks9 % 3) * QC


def _exp_in_ap(s_tile):
    """AP enumerating the 9 slots of a supertile in ks order, element-order
    compatible with a contiguous [128, 9*QC] output."""
    v = s_tile[:].rearrange("p (b s) -> p b s", s=512)[:, :, 0 : 3 * QC]
    return v.rearrange("p b (sl q) -> p b sl q", q=QC)


def build_nc(repeat=1):
    nc = bass.Bass()

    # ---- dram parameters ----
    xrgb_d = nc.declare_dram_parameter("xrgb", [C + 1, N], F32, isOutput=False)
    xq_d = nc.declare_dram_parameter("xq", [C, NQ], F32, isOutput=False)
    xdep_d = nc.declare_dram_parameter("xdep", [2, 576], F32, isOutput=False)
    qoff_d = nc.declare_dram_parameter("qoff", [1, 1], mybir.dt.uint32, isOutput=False)
    wk_r_d = nc.declare_dram_parameter("wk_r", [C, 2 * KT], F32, isOutput=False)
    wq_r_d = nc.declare_dram_parameter("wq_r", [C, 2 * KT], F32, isOutput=False)
    wk_d_d = nc.declare_dram_parameter("wk_d", [C, 2 * KT], F32, isOutput=False)
    wq_d_d = nc.declare_dram_parameter("wq_d", [C, 2 * KT], F32, isOutput=False)
    wvs_r_d = nc.declare_dram_parameter("wvs_r", [C + 1, 72], F32, isOutput=False)
    wvs_d_d = nc.declare_dram_parameter("wvs_d", [C + 1, 72], F32, isOutput=False)
    wexpb_d = nc.declare_dram_parameter("wexpb", [2, C], F32, isOutput=False)
    wf_r_d = nc.declare_dram_parameter("wf_r", [KT, 2 * C], F32, isOutput=False)
    wf_d_d = nc.declare_dram_parameter("wf_d", [KT, 2 * C], F32, isOutput=False)
    biasp_d = nc.declare_dram_parameter("biasp", [C, 1], F32, isOutput=False)
    e4_d = nc.declare_dram_parameter("e4", [4, KT], F32, isOutput=False)
    y_d = nc.declare_dram_parameter("y", [C, NQ], F32, isOutput=True)

    with _TileContext(nc) as tc:
        with (
            tc.tile_pool(name="const", bufs=1) as cpool,
            tc.tile_pool(name="work", bufs=1) as wpool,
        ):
            # ---- load inputs / weights to SBUF ----
            xrgb = cpool.tile([C + 1, N], F32)
            nc.sync.dma_start(xrgb[:], xrgb_d[:])
            xq = cpool.tile([C, NQ], F32)
            nc.sync.dma_start(xq[:], xq_d[:])
            xdep = cpool.tile([2, 576], F32)
            nc.sync.dma_start(xdep[:], xdep_d[:])
            qoff = cpool.tile([1, 1], mybir.dt.uint32)
            nc.sync.dma_start(qoff[:], qoff_d[:])
            wk = {}
            wqw = {}
            wvs = {}
            wf = {}
            for m, wk_src, wq_src, wvs_src, wf_src in (
                ("r", wk_r_d, wq_r_d, wvs_r_d, wf_r_d),
                ("d", wk_d_d, wq_d_d, wvs_d_d, wf_d_d),
            ):
                wk[m] = cpool.tile([C, 2 * KT], F32, tag=f"wk_{m}", name=f"wk_{m}")
                nc.sync.dma_start(wk[m][:], wk_src[:])
                wqw[m] = cpool.tile([C, 2 * KT], F32, tag=f"wq_{m}", name=f"wq_{m}")
                nc.sync.dma_start(wqw[m][:], wq_src[:])
                wvs[m] = cpool.tile([C + 1, 72], F32, tag=f"wvs_{m}", name=f"wvs_{m}")
                nc.sync.dma_start(wvs[m][:], wvs_src[:])
                wf[m] = cpool.tile([KT, 2 * C], F32, tag=f"wf_{m}", name=f"wf_{m}")
                nc.sync.dma_start(wf[m][:], wf_src[:])
            wexpb = cpool.tile([2, C], F32)
            nc.sync.dma_start(wexpb[:], wexpb_d[:])
            biasp = cpool.tile([C, 1], F32)
            nc.sync.dma_start(biasp[:], biasp_d[:])
            e4 = cpool.tile([4, KT], F32)
            nc.sync.dma_start(e4[:], e4_d[:])

            # ---- depth pipeline ----
            dep = cpool.tile([C + 1, N], F32)  # row 64 = ones
            nc.gpsimd.memset(dep[C : C + 1, :], 1.0)

            with tc.tile_pool(name="ppsum", bufs=2, space="PSUM") as ppool:
                # conv1x1 + bias (+relu below): [64, 576]
                dps = ppool.tile([C, 576], F32, tag="p576", bufs=1)
                nc.tensor.matmul(
                    dps[:, 0:512], wexpb[:, :], xdep[:, 0:512], start=True, stop=True
                )
                nc.tensor.matmul(
                    dps[:, 512:576], wexpb[:, :], xdep[:, 512:576], start=True, stop=True
                )
                Rm = wpool.tile([C, 576], F32)
                nc.scalar.activation(Rm[:], dps[:], AF.Relu)

                # upsample w': [64, 24, 24] -> A [64, 24, 48]
                A = wpool.tile([C, 24 * 48], F32)
                t75 = wpool.tile([C, 576], F32)
                t25 = wpool.tile([C, 576], F32)
                nc.vector.tensor_scalar_mul(t75[:], Rm[:], 0.75)
                nc.vector.tensor_scalar_mul(t25[:], Rm[:], 0.25)
                R3_75 = t75[:].rearrange("p (h w) -> p h w", w=24)
                R3_25 = t25[:].rearrange("p (h w) -> p h w", w=24)
                R3 = Rm[:].rearrange("p (h w) -> p h w", w=24)
                Av = A[:].rearrange("p (h j t) -> p h j t", j=24, t=2)
                nc.vector.tensor_add(
                    Av[:, :, 1:, 0], R3_75[:, :, 1:], R3_25[:, :, 0:23]
                )
                nc.vector.tensor_copy(Av[:, :, 0:1, 0], R3[:, :, 0:1])
                nc.vector.tensor_add(
                    Av[:, :, 0:23, 1], R3_75[:, :, 0:23], R3_25[:, :, 1:]
                )
                nc.vector.tensor_copy(Av[:, :, 23:24, 1], R3[:, :, 23:24])

                # upsample h': A [64, 24, 48] -> dep[0:64] as [64, 48, 48]
                u75 = wpool.tile([C, 24 * 48], F32)
                u25 = wpool.tile([C, 24 * 48], F32)
                nc.vector.tensor_scalar_mul(u75[:], A[:], 0.75)
                nc.vector.tensor_scalar_mul(u25[:], A[:], 0.25)
                A3 = A[:].rearrange("p (h w) -> p h w", w=48)
                A3_75 = u75[:].rearrange("p (h w) -> p h w", w=48)
                A3_25 = u25[:].rearrange("p (h w) -> p h w", w=48)
                Bv = dep[0:C, :].rearrange("p (i t w) -> p i t w", t=2, w=48)
                nc.vector.tensor_add(
                    Bv[:, 1:, 0, :], A3_75[:, 1:, :], A3_25[:, 0:23, :]
                )
                nc.vector.tensor_copy(Bv[:, 0:1, 0, :], A3[:, 0:1, :])
                nc.vector.tensor_add(
                    Bv[:, 0:23, 1, :], A3_75[:, 0:23, :], A3_25[:, 1:, :]
                )
                nc.vector.tensor_copy(Bv[:, 23:24, 1, :], A3[:, 23:24, :])

                # ---- per-core dep query slice (dynamic offset) ----
                dep_q = cpool.tile([C, NQ], F32)
                regs = nc.alloc_registers()
                nc.regs_load(regs, qoff[0:1, 0:1])
                q0v = nc.snap(regs, donate=True, min_val=0, max_val=N - NQ)
                nc.vector.tensor_copy(dep_q[:], dep[0:C, bass.ds(q0v, NQ)])

                # ---- projections into strip layout ----
                # Kt32[m][32j+d, g*N + n] = (w_k[m] @ x)[8*(4g+j)+d, n]
                # via host-permuted lhsT (junk rows get zero weight columns)
                kt = {}
                qt = {}
                vstar = {}
                for m, xin, qin in (("r", xrgb, xq), ("d", dep, dep_q)):
                    kt[m] = cpool.tile([KT, 2 * N], MM_DT, tag=f"kt_{m}", name=f"kt_{m}")
                    for g in range(2):
                        for c0 in range(0, N, 512):
                            cw = min(512, N - c0)
                            kp = ppool.tile([KT, 512], F32, tag="kp", name="kp")
                            nc.tensor.matmul(
                                kp[:, 0:cw],
                                wk[m][:, g * KT : (g + 1) * KT],
                                xin[0:C, c0 : c0 + cw],
                                start=True,
                                stop=True,
                            )
                            nc.vector.tensor_copy(
                                kt[m][:, g * N + c0 : g * N + c0 + cw], kp[:, 0:cw]
                            )
                    qt[m] = cpool.tile([KT, 2 * NQ], MM_DT, tag=f"qt_{m}", name=f"qt_{m}")
                    for g in range(2):
                        for c0 in range(0, NQ, 288):
                            qp_ = ppool.tile([KT, 512], F32, tag="kp", name="qp_")
                            nc.tensor.matmul(
                                qp_[:, 0:288],
                                wqw[m][:, g * KT : (g + 1) * KT],
                                qin[:, c0 : c0 + 288],
                                start=True,
                                stop=True,
                            )
                            nc.vector.tensor_copy(
                                qt[m][:, g * NQ + c0 : g * NQ + c0 + 288], qp_[:, 0:288]
                            )
                    vstar[m] = cpool.tile(
                        [KT, NKT * 72], MM_DT, tag=f"vs_{m}", name=f"vs_{m}"
                    )
                    for ks in range(NKT):
                        vp = ppool.tile([KT, 72], F32, tag="p72", name="vp")
                        nc.tensor.matmul(
                            vp[:],
                            xin[:, ks * KT : (ks + 1) * KT],
                            wvs[m][:, :],
                            start=True,
                            stop=True,
                        )
                        nc.vector.tensor_copy(
                            vstar[m][:, ks * 72 : (ks + 1) * 72], vp[:]
                        )

            # ---- attention ----
            import contextlib
            rep_ctx = tc.For_i(0, repeat, 1) if repeat > 1 else contextlib.nullcontext()
            with (
                tc.tile_pool(name="sa", bufs=1, space="PSUM") as sapool,
                tc.tile_pool(name="sb", bufs=1, space="PSUM") as sbpool,
                tc.tile_pool(name="av", bufs=2, space="PSUM") as avpool,
                tc.tile_pool(name="att", bufs=2) as apool,
                rep_ctx,
            ):
                for qc in range(NQC):
                    qsl = slice(qc * QC, (qc + 1) * QC)
                    xt = {}
                    for m, kv in (("r", "d"), ("d", "r")):
                        qm = qt[m]
                        km = kt[kv]
                        vm = vstar[kv]
                        xnum = apool.tile([KT, 2 * QC], F32, tag="xnum", name="xnum")
                        nc.vector.memset(xnum[:], 0.0)
                        av = avpool.tile([KT, 2 * QC], F32, tag="avt", name="av")
                        for h in range(H):
                            g, j = divmod(h, 4)
                            strip = 32 * j
                            pt = apool.tile([KT, NKT * QC], MM_DT, tag="pt", name="pt")
                            for half, spool in ((0, sapool), (1, sbpool)):
                                st = spool.tile(
                                    [KT, 1536], F32, tag=f"s{half}", name=f"s{half}"
                                )
                                for ks9 in range(9):
                                    ks = half * 9 + ks9
                                    o = _sa_off(ks9)
                                    nc.tensor.matmul(
                                        st[:, o : o + QC],
                                        km[strip : strip + 8, g * N + ks * KT : g * N + (ks + 1) * KT],
                                        qm[strip : strip + 8, g * NQ + qc * QC : g * NQ + (qc + 1) * QC],
                                        start=True,
                                        stop=True,
                                        tile_position=(strip, 0),
                                    )
                                nc.scalar.activation(
                                    pt[:, half * 9 * QC : (half + 1) * 9 * QC],
                                    _exp_in_ap(st),
                                    AF.Exp,
                                    scale=SCALE,
                                )
                            for ks in range(NKT):
                                nc.tensor.matmul(
                                    av[strip : strip + 9, g * QC : (g + 1) * QC],
                                    vm[:, ks * 72 + 9 * h : ks * 72 + 9 * h + 9],
                                    pt[:, ks * QC : (ks + 1) * QC],
                                    start=(ks == 0),
                                    stop=(ks == NKT - 1),
                                    tile_position=(0, strip),
                                    skip_group_check=True,
                                )
                            nc.vector.tensor_copy(
                                xnum[strip : strip + 9, g * QC : (g + 1) * QC],
                                av[strip : strip + 9, g * QC : (g + 1) * QC],
                            )
                        # denominators -> [4, 2*QC] (j on partitions, g in free)
                        dens4 = apool.tile([4, 2 * QC], F32, tag="dens4", name="dens4")
                        for h in range(H):
                            g, j = divmod(h, 4)
                            nc.sync.dma_start(
                                dens4[j : j + 1, g * QC : (g + 1) * QC],
                                xnum[32 * j + 8 : 32 * j + 9, g * QC : (g + 1) * QC],
                            )
                        recd = apool.tile([4, 2 * QC], F32, tag="recd", name="recd")
                        nc.vector.reciprocal(recd[:], dens4[:])
                        denx = avpool.tile([KT, 2 * QC], F32, tag="avt", name="denx")
                        nc.tensor.matmul(
                            denx[:], e4[:, :], recd[:], start=True, stop=True
                        )
                        xt[m] = apool.tile(
                            [KT, 2 * QC], F32, tag=f"xt_{m}", name=f"xt_{m}"
                        )
                        nc.vector.tensor_mul(xt[m][:], xnum[:], denx[:])
                    fp = avpool.tile([C, QC], F32, tag="avt", name="fp")
                    first = True
                    for m in ("r", "d"):
                        for g in range(2):
                            nc.tensor.matmul(
                                fp[:],
                                wf[m][:, g * C : (g + 1) * C],
                                xt[m][:, g * QC : (g + 1) * QC],
                                start=first,
                                stop=(m == "d" and g == 1),
                            )
                            first = False
                    outt = apool.tile([C, QC], F32, tag="outt", name="outt")
                    nc.scalar.activation(outt[:], fp[:], AF.Gelu, bias=biasp[:])
                    nc.sync.dma_start(y_d[:, qsl], outt[:])

    return nc


# ---------------- host side ----------------

_BUILT = {}


def _get_nc():
    if "nc" not in _BUILT:
        _BUILT["nc"] = build_nc()
    return _BUILT["nc"]


def _host_prep(inputs):
    """Build per-core input maps from full inputs."""
    f = lambda k: np.ascontiguousarray(np.asarray(inputs[k], np.float32))
    rgb_fea = f("rgb_fea")
    depth_fea = f("depth_fea")
    w_exp = f("w_exp")
    b_exp = f("b_exp")

    def vstar_w(w_v):
        W = np.zeros((C + 1, 72), np.float32)
        for h in range(H):
            W[0:C, 9 * h : 9 * h + 8] = w_v.T[:, 8 * h : 8 * h + 8]
            W[C, 9 * h + 8] = 1.0
        return np.ascontiguousarray(W)

    def fuse_w(Wp):
        # Wp [64 out, 64 in]; strip layout rows 32j+d = in-channel 8*(4g+j)+d
        W = np.zeros((KT, 2 * C), np.float32)
        for g in range(2):
            for j in range(4):
                h = 4 * g + j
                W[32 * j : 32 * j + 8, g * C : (g + 1) * C] = Wp[:, 8 * h : 8 * h + 8].T
        return np.ascontiguousarray(W)

    w_comp = f("w_comp")
    W_r, W_d = w_comp[:, :C], w_comp[:, C:]
    e4 = np.zeros((4, KT), np.float32)
    for j in range(4):
        e4[j, 32 * j : 32 * j + 8] = 1.0
    def strip_w(w):
        # lhsT [64 in, 2*128]: col g*128 + 32j+d = row 8*(4g+j)+d of w
        W = np.zeros((C, 2 * KT), np.float32)
        for g in range(2):
            for j in range(4):
                h = 4 * g + j
                W[:, g * KT + 32 * j : g * KT + 32 * j + 8] = w[8 * h : 8 * h + 8, :].T
        return np.ascontiguousarray(W)

    shared = {
        "wk_r": strip_w(f("w_rgb_k")),
        "wq_r": strip_w(f("w_rgb_q")),
        "wk_d": strip_w(f("w_dep_k")),
        "wq_d": strip_w(f("w_dep_q")),
        "wvs_r": vstar_w(f("w_rgb_v")),
        "wvs_d": vstar_w(f("w_dep_v")),
        "wexpb": np.ascontiguousarray(
            np.stack([w_exp.ravel(), b_exp.ravel()]).astype(np.float32)
        ),
        "wf_r": fuse_w(W_r @ f("w_rgb_proj")),
        "wf_d": fuse_w(W_d @ f("w_dep_proj")),
        "biasp": np.ascontiguousarray(
            (W_r @ f("b_rgb_proj") + W_d @ f("b_dep_proj") + f("b_comp"))[:, None]
        ),
        "e4": e4,
    }
    ones = np.ones((1, N), np.float32)
    in_maps = []
    for core in range(8):
        b, qp = divmod(core, 4)
        xrgb = np.ascontiguousarray(np.vstack([rgb_fea[b].reshape(C, N), ones]))
        m = dict(shared)
        m["xrgb"] = xrgb
        m["xq"] = np.ascontiguousarray(xrgb[0:C, qp * NQ : (qp + 1) * NQ])
        m["xdep"] = np.ascontiguousarray(
            np.vstack(
                [depth_fea[b, 0].reshape(1, 576), np.ones((1, 576), np.float32)]
            )
        )
        m["qoff"] = np.array([[qp * NQ]], dtype=np.uint32)
        in_maps.append(m)
    return in_maps


def _assemble(results):
    out = np.zeros((2, C, 48, 48), np.float32)
    for core in range(8):
        b, qp = divmod(core, 4)
        y = results[core]["y"]
        out[b, :, qp * 12 : (qp + 1) * 12, :] = y.reshape(C, 12, 48)
    # (c, h, w) -> reference order (c, w, h)
    return np.ascontiguousarray(out.transpose(0, 1, 3, 2))


def kernel(**inputs):
    nc = _get_nc()
    in_maps = _host_prep(inputs)
    res = run_bass_kernel_spmd(nc, in_maps, list(range(8)))
    return _assemble(res.results)


def run_sim_core(inputs, core=0):
    """CoreSim single-core debug path (not used by the harness)."""
    from concourse import bass_interp

    nc = build_nc()
    sim = bass_interp.CoreSim(nc)
    in_map = _host_prep(inputs)[core]
    for k, v in in_map.items():
        sim.tensor(k)[:] = v
    sim.simulate()
    return np.array(sim.tensor("y"))
